# revision 21
# baseline (speedup 1.0000x reference)
"""Trainium2 Bass kernel for nn_MessagePassingNN, v3.

Key changes over v2 (see prep_v2.py for layout):
  - A-term transport in fp8e4m3, position-major shard tables split into
    lo (blocks 0..49) / hi (50..97) halves. Two AllGathers per iteration,
    each launched as soon as its half's AC writes complete, so the
    collectives overlap compute (they were 100% exposed in v2: 2.3ms).
  - dma_gather fetches 256B elements = two adjacent position rows; tiles
    have uniform position parity so the X matmul just reads the right
    128-col half of the gathered pair.
  - Tiles binned by (dest block, src half, parity); message padding
    reduced via degree-aware packing + global half/parity balancing.
  - Iteration split into phase L (src-lo tiles -> partial aggs staged in
    SBUF) and phase H (src-hi tiles + restore + GRU + next AC).
  - hT kept in bf16 (drops per-block h copies), gather calls batched to
    2048 indices (descriptor-gen overhead halved).
"""
import sys

sys.path.insert(0, "/opt/trn_rl_repo")

import numpy as np
import ml_dtypes

import concourse.bass as bass
import concourse.bacc as bacc
import concourse.mybir as mybir
import concourse.tile as tile
import concourse.bass_utils as bass_utils
from concourse.masks import make_identity

"""Preprocessing v2 for the fp8 / 2-phase-pipelined GNN kernel.

Layout (per core, SH=12500 nodes, 98 blocks x 128 positions):
  Lo half = blocks 0..49, hi = blocks 50..97 (chosen by a balance pass).
  fp8 shard tables, position-major: shard_lo [50*128=6400 rows, 128 cols],
  shard_hi [6144, 128]; row (b*128+p) = A[node at (b, p)] in fp8.
  Replicas (AllGather out): rep_lo [51200, 128], rep_hi [49152, 128],
  viewed for gather as [25600, 256] / [24576, 256]: one 256B element covers
  two adjacent positions (2u, 2u+1).
  Gather idx for a message with source (o, b, p):
    lo: o*3200 + b*64 + p//2   hi: o*3072 + (b-50)*64 + p//2   (< 25600)
  The message's A row is the (p%2) half of the gathered element; tiles are
  sorted by (parity=p%2, idx) within each (dest block, h) run so the X
  matmul splits at one partition boundary k (ident column slices).
"""

N_NODES = 100000
N_CORES = 8
SH = N_NODES // N_CORES          # 12500
BLK = 128
NBLK = 98
SHP = NBLK * BLK                 # 12544
H = 128
LOB = 50                         # lo blocks per core
HIB = NBLK - LOB                 # 48
LO_ROWS = LOB * BLK              # 6400 (positions)
HI_ROWS = HIB * BLK              # 6144
LO_UNITS = LO_ROWS // 2          # 3200 gather units per core
HI_UNITS = HI_ROWS // 2
GB = 7                           # blocks per gather group
NGRP = NBLK // GB                # 14
MAX_CALL = 1024                  # idx per dma_gather call


def _make_targets():
    """Per-(final block, h, parity) load targets aligned just under
    128-multiples (2-tile bins target 221, 3-tile bins 349; ~35 margin).
    Even final blocks: (lo, hi) = (442, 570); odd: (570, 442).
    Per-block total 1012 everywhere."""
    tgt = np.empty((NBLK, 2, 2), np.int64)
    for b in range(NBLK):
        big_h = 1 if b % 2 == 0 else 0
        for h in range(2):
            tgt[b, h, 0] = 221
            tgt[b, h, 1] = 349 if h == big_h else 221
    return tgt


def _pack_core(deg, capP):
    """Assign SH nodes to 98 blocks balancing per-block in-edge counts
    against the target pattern. Returns pos[n] in [0, SHP)."""
    slots = np.full(NBLK, BLK, np.int64)
    slots[-1] = SH - BLK * (NBLK - 1)                # 84
    fill = np.zeros(NBLK, np.int64)
    order = np.argsort(-deg, kind="stable")
    assign = np.full(SH, -1, np.int64)
    BIG = 1 << 40
    for n in order:
        d = deg[n]
        rem = capP - fill - d
        over = np.where(rem < 0, -rem, 0)
        score = over * 4096 - rem
        score = np.where(slots <= 0, BIG, score)
        b = int(np.argmin(score))
        fill[b] += d
        slots[b] -= 1
        assign[n] = b
    pos = np.full(SH, -1, np.int64)
    nxt = np.zeros(NBLK, np.int64)
    for n in range(SH):
        b = assign[n]
        pos[n] = b * BLK + nxt[b]
        nxt[b] += 1
    return pos


def _balance_halves(first, second, pos_list, sweeps=4):
    """Pick per-core which 50 blocks are 'lo' (25 with lo-target 442, 25
    with 570; hi: 24+24) so per-(dest core, dest block) message counts
    split near their targets. Each block is assigned a target class
    cls in {0: lo-frac 442/1012, 1: 570/1012} and a final index with
    matching parity. Returns per-core permutations + per-core block cls."""
    owner_d = second // SH
    owner_s = first // SH
    src_loc = first % SH
    dst_loc = second % SH
    src_blk = np.empty(len(first), np.int64)
    dst_blk = np.empty(len(first), np.int64)
    for c in range(N_CORES):
        m = owner_s == c
        src_blk[m] = pos_list[c][src_loc[m]] // BLK
        m = owner_d == c
        dst_blk[m] = pos_list[c][dst_loc[m]] // BLK
    bin_id = owner_d * NBLK + dst_blk
    NB = N_CORES * NBLK
    src_key = owner_s * NBLK + src_blk
    tot = np.bincount(bin_id, minlength=NB)

    contrib = {}
    order_sb = np.argsort(src_key, kind="stable")
    sk_sorted = src_key[order_sb]
    bounds = np.searchsorted(sk_sorted, np.arange(N_CORES * NBLK + 1))
    for c in range(N_CORES):
        for b in range(NBLK):
            k = c * NBLK + b
            sl = order_sb[bounds[k]:bounds[k + 1]]
            bins, cnts = np.unique(bin_id[sl], return_counts=True)
            contrib[(c, b)] = (bins, cnts.astype(np.float64))

    # target class per packed block: 0 -> lo target 442/1012, 1 -> 570/1012
    cls = np.tile(np.arange(NBLK) % 2, (N_CORES, 1))
    frac = np.array([442.0 / 1012.0, 570.0 / 1012.0])
    cls_of_bin = np.concatenate([cls[c] for c in range(N_CORES)])
    target = tot.astype(np.float64) * frac[cls_of_bin]

    half = [np.zeros(NBLK, np.int64) for _ in range(N_CORES)]
    for c in range(N_CORES):
        half[c][:LOB] = 1
    is_lo_flat = np.concatenate(half)[src_key]
    lo = np.bincount(bin_id, weights=is_lo_flat, minlength=NB)
    LO_CAP, HI_CAP = (25, 25), (24, 24)
    for _ in range(sweeps):
        for c in range(N_CORES):
            for b in range(NBLK):
                if half[c][b]:
                    bins, cnts = contrib[(c, b)]
                    lo[bins] -= cnts
            half[c][:] = 0
            w = sorted(range(NBLK),
                       key=lambda b: -contrib[(c, b)][1].sum())
            nlo = [0, 0]
            nhi = [0, 0]
            for b in w:
                k = cls[c][b]
                bins, cnts = contrib[(c, b)]
                d = lo[bins] - target[bins]
                # marginal squared-deviation: lo better iff sum(d*c) < 0
                score = np.sum(d * cnts)
                if (score < 0 and nlo[k] < LO_CAP[k]) or nhi[k] >= HI_CAP[k]:
                    half[c][b] = 1
                    lo[bins] += cnts
                    nlo[k] += 1
                else:
                    nhi[k] += 1
    perms = []
    for c in range(N_CORES):
        perm = np.empty(NBLK, np.int64)
        nxt = {(1, 0): 0, (1, 1): 1, (0, 0): LOB, (0, 1): LOB + 1}
        for b in range(NBLK):
            key = (int(half[c][b]), int(cls[c][b]))
            perm[b] = nxt[key]
            nxt[key] += 2
        assert perm.max() < NBLK
        perms.append(perm)
    return perms


def _assign_parity(first, second, pos_list, tgt):
    """Assign position parity (even/odd within block) per node so that
    per-(dest core, dest block, src half) bins split into even/odd parts
    under quantized caps from the target pattern. Returns per-core parity
    arrays [SH] in {0,1}; caller rebuilds pos. tgt: [NBLK, 2, 2] loads."""
    owner_d = second // SH
    owner_s = first // SH
    src_loc = first % SH
    dst_loc = second % SH
    src_blk = np.empty(len(first), np.int64)
    dst_blk = np.empty(len(first), np.int64)
    for c in range(N_CORES):
        m = owner_s == c
        src_blk[m] = pos_list[c][src_loc[m]] // BLK
        m = owner_d == c
        dst_blk[m] = pos_list[c][dst_loc[m]] // BLK
    s_h = (src_blk >= LOB).astype(np.int64)
    # bin per edge: (dest core, dest block, src h)
    bin_id = (owner_d * NBLK + dst_blk) * 2 + s_h
    NB = N_CORES * NBLK * 2
    cap_e = np.zeros(NB, np.int64)
    cap_o = np.zeros(NB, np.int64)
    for co in range(N_CORES):
        for b in range(NBLK):
            for h in range(2):
                k = (co * NBLK + b) * 2 + h
                cap_e[k] = -(-int(tgt[b, h, 0]) // BLK) * BLK
                cap_o[k] = -(-int(tgt[b, h, 1]) // BLK) * BLK
    # per source node: its out-edge bin contributions
    src_key = owner_s * SH + src_loc
    order = np.argsort(src_key, kind="stable")
    sk = src_key[order]
    bounds = np.searchsorted(sk, np.arange(N_CORES * SH + 1))
    cnt_e = np.zeros(NB, np.float64)
    cnt_o = np.zeros(NB, np.float64)
    parity = [np.zeros(SH, np.int64) for _ in range(N_CORES)]
    # per (core, block): even/odd slot capacity from actual node counts
    evcap = np.zeros((N_CORES, NBLK), np.int64)
    odcap = np.zeros((N_CORES, NBLK), np.int64)
    for c in range(N_CORES):
        cntb = np.bincount(pos_list[c] // BLK, minlength=NBLK)
        evcap[c] = (cntb + 1) // 2
        odcap[c] = cntb // 2
    # process nodes in descending out-degree order; 2nd pass refines
    deg = bounds[1:] - bounds[:-1]
    node_order = np.argsort(-deg, kind="stable")
    for pss in range(2):
        for nk in node_order:
            c, n = nk // SH, nk % SH
            b = pos_list[c][n] // BLK
            sl = order[bounds[nk]:bounds[nk + 1]]
            if pss == 1:
                # remove current assignment
                if len(sl):
                    bins0, cnts0 = np.unique(bin_id[sl], return_counts=True)
                    if parity[c][n] == 0:
                        cnt_e[bins0] -= cnts0
                    else:
                        cnt_o[bins0] -= cnts0
                if parity[c][n] == 0:
                    evcap[c, b] += 1
                else:
                    odcap[c, b] += 1
                parity[c][n] = 0
            if len(sl) == 0:
                if evcap[c, b] > 0 and (evcap[c, b] >= odcap[c, b]
                                        or odcap[c, b] == 0):
                    evcap[c, b] -= 1
                else:
                    odcap[c, b] -= 1
                    parity[c][n] = 1
                continue
            bins, cnts = np.unique(bin_id[sl], return_counts=True)
            cf = cnts.astype(np.float64)
            ce, co = cap_e[bins].clip(1), cap_o[bins].clip(1)
            fe = cnt_e[bins] / ce
            fo = cnt_o[bins] / co
            ove = (np.maximum(cnt_e[bins] + cf - ce, 0)
                   - np.maximum(cnt_e[bins] - ce, 0))
            ovo = (np.maximum(cnt_o[bins] + cf - co, 0)
                   - np.maximum(cnt_o[bins] - co, 0))
            cost_e = (cf * (fe - fo)).sum() + 1000.0 * ove.sum()
            cost_o = (cf * (fo - fe)).sum() + 1000.0 * ovo.sum()
            pick_e = cost_e <= cost_o
            if evcap[c, b] <= 0:
                pick_e = False
            if odcap[c, b] <= 0:
                pick_e = True
            if pick_e:
                cnt_e[bins] += cf
                evcap[c, b] -= 1
            else:
                cnt_o[bins] += cf
                odcap[c, b] -= 1
                parity[c][n] = 1
    return parity


def preprocess(features, first, second, graph_ids):
    first = np.asarray(first, np.int64)
    second = np.asarray(second, np.int64)
    graph_ids = np.asarray(graph_ids, np.int64)
    features = np.asarray(features, np.float32)

    owner_s = first // SH
    owner_d = second // SH
    src_loc = first % SH
    dst_loc = second % SH

    tgt = _make_targets()                               # [NBLK, 2, 2]
    capP = tgt.sum(axis=(1, 2))                         # 1012 per block

    pos_list = []
    for c in range(N_CORES):
        m = owner_d == c
        deg = np.bincount(dst_loc[m], minlength=SH)
        pos_list.append(_pack_core(deg, capP))

    perms = _balance_halves(first, second, pos_list)
    for c in range(N_CORES):
        b_old = pos_list[c] // BLK
        pos_list[c] = perms[c][b_old] * BLK + pos_list[c] % BLK

    # parity assignment, then rebuild positions: parity fixes p % 2
    parity = _assign_parity(first, second, pos_list, tgt)
    for c in range(N_CORES):
        b = pos_list[c] // BLK
        newpos = np.full(SH, -1, np.int64)
        for bb in range(NBLK):
            m = b == bb
            par = parity[c][m]
            nodes = np.flatnonzero(m)
            ev = nodes[par[np.arange(len(nodes))] == 0] \
                if False else nodes[parity[c][nodes] == 0]
            od = nodes[parity[c][nodes] == 1]
            newpos[ev] = bb * BLK + 2 * np.arange(len(ev))
            newpos[od] = bb * BLK + 2 * np.arange(len(od)) + 1
        pos_list[c] = newpos

    pos_all = np.concatenate([o * SHP + pos_list[o] for o in range(N_CORES)])
    spos = pos_all[first]
    s_o = spos // SHP
    s_r = spos % SHP
    s_b = s_r // BLK
    s_p = s_r % BLK
    s_h = (s_b >= LOB).astype(np.int64)
    s_par = s_p % 2
    idx_lo = s_o * LO_UNITS + s_b * 64 + s_p // 2
    idx_hi = s_o * HI_UNITS + (s_b - LOB) * 64 + s_p // 2
    s_idx = np.where(s_h == 0, idx_lo, idx_hi)
    assert s_idx.max() < 32768

    per_core = []
    counts = np.zeros((N_CORES, NBLK, 2, 2), np.int64)
    for c in range(N_CORES):
        m = owner_d == c
        loc = pos_list[c][dst_loc[m]]
        blk = loc // BLK
        h = s_h[m]
        par = s_par[m]
        si = s_idx[m]
        order = np.lexsort((si, par, blk, h))
        per_core.append((loc[order], blk[order], h[order], par[order],
                         si[order]))
        np.add.at(counts[c], (blk, h, par), 1)

    P = np.maximum(1, -(-counts.max(axis=0) // BLK))   # [NBLK, 2, 2]

    groups = [list(range(g * GB, (g + 1) * GB)) for g in range(NGRP)]
    slot_base = np.zeros((NBLK, 2, 2), np.int64)
    tile_list = []
    call_list = []                 # (h, slot0, n_idx, group)
    s0 = 0
    for h in range(2):
        for gi, blocks in enumerate(groups):
            c0 = s0
            for b in blocks:
                for par in range(2):
                    slot_base[b, h, par] = s0
                    for _ in range(int(P[b, h, par])):
                        tile_list.append((b, h, par))
                        s0 += BLK
            csz = s0 - c0
            while csz > 0:
                take = min(csz, MAX_CALL)
                call_list.append((h, c0, take, gi))
                c0 += take
                csz -= take
    LC = s0
    TT = LC // BLK
    tile_block = np.array([t[0] for t in tile_list], np.int64)
    tile_h = np.array([t[1] for t in tile_list], np.int64)
    tile_par = np.array([t[2] for t in tile_list], np.int64)

    out = dict(P=P, groups=groups, LC=LC, TT=TT, tile_block=tile_block,
               tile_h=tile_h, tile_par=tile_par, call_list=call_list,
               slot_base=slot_base, pos_list=pos_list)
    idx16_l, indT_l, indm_l, h0T_l, gid_l = [], [], [], [], []
    for c in range(N_CORES):
        loc, blk, h, par, si = per_core[c]
        seg = np.full(LC, -1.0, np.float32)
        gidx = np.zeros(LC, np.int64)
        n = len(blk)
        run_change = np.ones(n, bool)
        run_change[1:] = ((blk[1:] != blk[:-1]) | (h[1:] != h[:-1])
                          | (par[1:] != par[:-1]))
        run_start = np.maximum.accumulate(
            np.where(run_change, np.arange(n), 0))
        within = np.arange(n) - run_start
        slot = slot_base[blk, h, par] + within
        seg[slot] = (loc - blk * BLK).astype(np.float32)
        gidx[slot] = si
        idx16 = gidx.astype(np.int16)
        idxw = np.zeros((16, LC // 16), np.int16)
        for hh, c0, take, gi in call_list:
            idxw[:, c0 // 16:(c0 + take) // 16] = (
                idx16[c0:c0 + take].reshape(take // 16, 16).T)
        idx_full = np.ascontiguousarray(np.tile(idxw, (8, 1)))
        segs = seg.reshape(TT, 1, BLK)
        dd = np.arange(BLK, dtype=np.float32).reshape(1, BLK, 1)
        indT = (segs == dd).astype(ml_dtypes.float8_e4m3)   # [TT, dest, msg]
        indT_h = np.ascontiguousarray(indT.transpose(1, 0, 2).reshape(BLK, LC))
        indm = indT.transpose(0, 2, 1)
        indm_h = np.ascontiguousarray(indm.transpose(1, 0, 2).reshape(BLK, LC))
        idx16_l.append(idx_full)
        indT_l.append(indT_h)
        indm_l.append(indm_h)
        h0 = np.zeros((SHP, H), np.float32)
        h0[pos_list[c]] = features[c * SH:(c + 1) * SH]
        h0T_l.append(np.ascontiguousarray(h0.T).astype(ml_dtypes.bfloat16))
        gid = np.full(SHP, -1.0, np.float32)
        gid[pos_list[c]] = graph_ids[c * SH:(c + 1) * SH].astype(np.float32)
        gid_l.append(np.ascontiguousarray(gid.reshape(NBLK, BLK).T))
    out.update(idx16=idx16_l, indT=indT_l, indm=indm_l,
               h0T=h0T_l, gid=gid_l)
    return out


# ---- preprocessing (inlined from prep_v2) ----

F32 = mybir.dt.float32
BF16 = mybir.dt.bfloat16
F8 = mybir.dt.float8e4
I16 = mybir.dt.int16

T_ITERS = 8
G = 512
RU = 256

LAM = 1.0507009873554805
ALPHA = 1.6732632423543772
LA = LAM * ALPHA
LNLA = float(np.log(LA))

AG_GROUPS = [list(range(N_CORES))]


def _build_program(meta, b3_val, t_iters=T_ITERS):
    LC = meta["LC"]
    TT = meta["TT"]
    groups = meta["groups"]
    tile_block = meta["tile_block"]
    tile_h = meta["tile_h"]
    tile_par = meta["tile_par"]
    call_list = meta["call_list"]

    # per (phase h, group): first tile index, tile count
    tile_of_slot = np.arange(TT)
    gmeta = {}
    pos = 0
    for h in range(2):
        for gi in range(NGRP):
            calls = [cl for cl in call_list if cl[0] == h and cl[3] == gi]
            n = sum(cl[2] for cl in calls) // BLK
            gmeta[(h, gi)] = (pos, n, calls)
            pos += n
    assert pos == TT

    nc = bacc.Bacc(
        "TRN2",
        target_bir_lowering=False,
        debug=False,
        enable_asserts=False,
        num_devices=N_CORES,
        dynamic_dma_scratch_size=16384,
    )

    h0T_in = nc.dram_tensor("h0T", [BLK, SHP], BF16, kind="ExternalInput")
    idx_in = nc.dram_tensor("idx16", [BLK, LC // 16], I16, kind="ExternalInput")
    indT_in = nc.dram_tensor("indT", [BLK, LC], F8, kind="ExternalInput")
    indm_in = nc.dram_tensor("indm", [BLK, LC], F8, kind="ExternalInput")
    gid_in = nc.dram_tensor("gid", [BLK, NBLK], F32, kind="ExternalInput")
    wm1_in = nc.dram_tensor("wm1", [H, H], F32, kind="ExternalInput")
    wm2_in = nc.dram_tensor("wm2", [H, H], F32, kind="ExternalInput")
    bmr_in = nc.dram_tensor("bmr", [1, H], F32, kind="ExternalInput")
    wk_in = nc.dram_tensor("wk", [H, 3 * H], F32, kind="ExternalInput")
    uk_in = nc.dram_tensor("uk", [H, 3 * H], F32, kind="ExternalInput")
    bhh2_in = nc.dram_tensor("bhh2", [1, H], F32, kind="ExternalInput")
    bkc_in = nc.dram_tensor("bkc", [BLK, 3], F32, kind="ExternalInput")
    w1_in = nc.dram_tensor("w1", [H, RU], F32, kind="ExternalInput")
    w2_in = nc.dram_tensor("w2", [RU, RU], F32, kind="ExternalInput")
    w3_in = nc.dram_tensor("w3", [RU, 1], F32, kind="ExternalInput")
    b1r_in = nc.dram_tensor("b1r", [BLK, 2], F32, kind="ExternalInput")
    b1e_in = nc.dram_tensor("b1e", [BLK, 2], F32, kind="ExternalInput")
    b2r_in = nc.dram_tensor("b2r", [BLK, 2], F32, kind="ExternalInput")
    b2e_in = nc.dram_tensor("b2e", [BLK, 2], F32, kind="ExternalInput")
    out_dram = nc.dram_tensor("out", [1, G], F32, kind="ExternalOutput")

    with tile.TileContext(nc) as tc:
        with (
            tc.tile_pool(name="const", bufs=1) as cp,
            tc.tile_pool(name="gmp", bufs=2) as gmp,
            tc.tile_pool(name="indp", bufs=2) as indp,
            tc.tile_pool(name="sp", bufs=4) as spool,
            tc.tile_pool(name="wp", bufs=2) as wp,
            tc.tile_pool(name="final", bufs=1) as fp,
            tc.tile_pool(name="ps_x", bufs=2, space="PSUM") as psX,
            tc.tile_pool(name="ps_agg", bufs=2, space="PSUM") as psA,
            tc.tile_pool(name="ps_gru", bufs=2, space="PSUM") as psG,
            tc.tile_pool(name="ps_ac", bufs=2, space="PSUM") as psC,
            tc.tile_pool(name="dram", bufs=1, space="DRAM") as dp,
        ):
            ident_b = cp.tile([BLK, BLK], BF16)
            make_identity(nc, ident_b[:])
            ident8 = cp.tile([BLK, BLK], F8)
            nc.vector.tensor_copy(ident8[:], ident_b[:])
            iota_g = cp.tile([BLK, G], F32)
            lnla_c = cp.tile([BLK, 1], F32)
            nc.gpsimd.memset(lnla_c[:], LNLA)
            ones1_b = cp.tile([1, BLK], BF16)
            nc.gpsimd.memset(ones1_b[:], 1.0)

            iog_i = wp.tile([BLK, G], mybir.dt.int32, tag="iogi")
            nc.gpsimd.iota(iog_i[:], pattern=[[1, G]], base=0,
                           channel_multiplier=0)
            nc.vector.tensor_copy(iota_g[:], iog_i[:])
            hT = cp.tile([BLK, SHP], BF16)
            nc.sync.dma_start(hT[:], h0T_in[:])
            idx_sb = cp.tile([BLK, LC // 16], I16)
            nc.sync.dma_start(idx_sb[:], idx_in[:])
            gid_sb = cp.tile([BLK, NBLK], F32)
            nc.sync.dma_start(gid_sb[:], gid_in[:])

            def load_bf(t_in, shape, tag):
                t32 = wp.tile(shape, F32, tag="ldf32")
                nc.sync.dma_start(t32[:], t_in[:])
                tb = cp.tile(shape, BF16, tag=tag)
                nc.vector.tensor_copy(tb[:], t32[:])
                return tb

            wm1 = load_bf(wm1_in, [H, H], "wm1")
            wm2 = load_bf(wm2_in, [H, H], "wm2")
            wk = load_bf(wk_in, [H, 3 * H], "wk")
            uk = load_bf(uk_in, [H, 3 * H], "uk")
            w1 = load_bf(w1_in, [H, RU], "w1")
            bmr_b = load_bf(bmr_in, [1, H], "bmr")
            bhh2 = load_bf(bhh2_in, [1, H], "bhh2")
            bkc = cp.tile([BLK, 3], F32)
            nc.sync.dma_start(bkc[:], bkc_in[:])
            w2q = []
            for i in range(2):
                row = []
                for j in range(2):
                    t32 = wp.tile([BLK, BLK], F32, tag="ldw2")
                    nc.sync.dma_start(
                        t32[:], w2_in[bass.ts(i, BLK), bass.ts(j, BLK)])
                    tb = cp.tile([BLK, BLK], BF16, tag=f"w2q{i}{j}")
                    nc.vector.tensor_copy(tb[:], t32[:])
                    row.append(tb)
                w2q.append(row)
            w3ab = []
            for i in range(2):
                t32 = wp.tile([BLK, 1], F32, tag="ldw3")
                nc.sync.dma_start(t32[:], w3_in[bass.ts(i, BLK), :])
                tb = cp.tile([BLK, 1], BF16, tag=f"w3{i}")
                nc.vector.tensor_copy(tb[:], t32[:])
                w3ab.append(tb)
            w3a, w3b = w3ab
            b1r = cp.tile([BLK, 2], F32)
            nc.sync.dma_start(b1r[:], b1r_in[:])
            b1e = cp.tile([BLK, 2], F32)
            nc.sync.dma_start(b1e[:], b1e_in[:])
            b2r = cp.tile([BLK, 2], F32)
            nc.sync.dma_start(b2r[:], b2r_in[:])
            b2e = cp.tile([BLK, 2], F32)
            nc.sync.dma_start(b2e[:], b2e_in[:])
            b3c = cp.tile([1, 1], F32)
            nc.gpsimd.memset(b3c[:], float(b3_val))

            c_all = cp.tile([BLK, NBLK * H], BF16)
            partial = cp.tile([BLK, NBLK * H], BF16)

            shard_lo = dp.tile([LO_UNITS, 256], F8)
            shard_hi = dp.tile([HI_UNITS, 256], F8)
            repLoA = dp.tile([N_CORES * LO_UNITS, 256], F8)
            repLoB = dp.tile([N_CORES * LO_UNITS, 256], F8)
            repHiA = dp.tile([N_CORES * HI_UNITS, 256], F8)
            repHiB = dp.tile([N_CORES * HI_UNITS, 256], F8)
            repLo = [repLoA, repLoB]
            repHi = [repHiA, repHiB]
            pool_in = dp.tile([BLK, G], F32)
            pool_out = dp.tile([BLK, G], F32)

            def ag_issue(h, t):
                # collectives run on the bf16 view (same bytes); fp8-typed
                # collectives are not exercised elsewhere, don't risk them
                if h == 0:
                    nc.gpsimd.collective_compute(
                        "AllGather", mybir.AluOpType.bypass,
                        replica_groups=AG_GROUPS,
                        ins=[shard_lo[:].bitcast(BF16).opt()],
                        outs=[repLo[t % 2][:].bitcast(BF16).opt()])
                else:
                    nc.gpsimd.collective_compute(
                        "AllGather", mybir.AluOpType.bypass,
                        replica_groups=AG_GROUPS,
                        ins=[shard_hi[:].bitcast(BF16).opt()],
                        outs=[repHi[t % 2][:].bitcast(BF16).opt()])

            def ac_block(b, t_next):
                ac_ps = psC.tile([BLK, 2 * H], F32, space="PSUM", tag="ac")
                hslice = hT[:, bass.ts(b, BLK)]
                nc.tensor.matmul(ac_ps[:, 0:H], lhsT=hslice, rhs=wm1[:],
                                 start=True, stop=True)
                nc.tensor.matmul(ac_ps[:, H:2 * H], lhsT=hslice, rhs=wm2[:],
                                 start=True, stop=False)
                nc.tensor.matmul(ac_ps[:, H:2 * H], lhsT=ones1_b[:],
                                 rhs=bmr_b[:], start=False, stop=True)
                a8 = wp.tile([BLK, H], F8, tag="a8")
                nc.vector.tensor_copy(a8[:], ac_ps[:, 0:H])
                if b < LOB:
                    nc.sync.dma_start(
                        shard_lo[b * 64:(b + 1) * 64, :], a8[:])
                else:
                    bb = b - LOB
                    nc.sync.dma_start(
                        shard_hi[bb * 64:(bb + 1) * 64, :], a8[:])
                nc.vector.tensor_copy(c_all[:, bass.ts(b, H)],
                                      ac_ps[:, H:2 * H])
                # collective launch points
                if b == LOB - 1:
                    ag_issue(0, t_next)
                elif b == NBLK - 1:
                    ag_issue(1, t_next)

            def gru_block(b, agg_ap, t):
                aggb = wp.tile([BLK, BLK], BF16, tag="aggb")
                nc.vector.tensor_copy(aggb[:], agg_ap)
                hslice = hT[:, bass.ts(b, BLK)]
                gps = psG.tile([BLK, 4 * H], F32, space="PSUM", tag="gru")
                nc.tensor.matmul(gps[:, 0:H], lhsT=wk[:, 0:H], rhs=aggb[:],
                                 start=True, stop=False)
                nc.tensor.matmul(gps[:, 0:H], lhsT=uk[:, 0:H], rhs=hslice,
                                 start=False, stop=True)
                nc.tensor.matmul(gps[:, H:2 * H], lhsT=wk[:, H:2 * H],
                                 rhs=aggb[:], start=True, stop=False)
                nc.tensor.matmul(gps[:, H:2 * H], lhsT=uk[:, H:2 * H],
                                 rhs=hslice, start=False, stop=True)
                nc.tensor.matmul(gps[:, 2 * H:3 * H],
                                 lhsT=wk[:, 2 * H:3 * H], rhs=aggb[:],
                                 start=True, stop=True)
                # gps_hh' = 0.5*(Uk_h @ h + bhh)   (uk pre-scaled by 0.5)
                nc.tensor.matmul(gps[:, 3 * H:4 * H],
                                 lhsT=uk[:, 2 * H:3 * H], rhs=hslice,
                                 start=True, stop=False)
                nc.tensor.matmul(gps[:, 3 * H:4 * H], lhsT=ones1_b[:],
                                 rhs=bhh2[:], start=False, stop=True)
                zT = wp.tile([BLK, BLK], BF16, tag="zT")
                nc.scalar.activation(zT[:], gps[:, 0:H],
                                     mybir.ActivationFunctionType.Tanh,
                                     bias=bkc[:, 0:1], scale=0.5)
                rT = wp.tile([BLK, BLK], BF16, tag="rT")
                nc.scalar.activation(rT[:], gps[:, H:2 * H],
                                     mybir.ActivationFunctionType.Tanh,
                                     bias=bkc[:, 1:2], scale=0.5)
                # u = (1 + rT) * gps_hh'  = r*(mhh+bhh)
                u1 = wp.tile([BLK, BLK], F32, tag="u1")
                nc.vector.scalar_tensor_tensor(
                    out=u1[:], in0=rT[:], scalar=1.0,
                    in1=gps[:, 3 * H:4 * H],
                    op0=mybir.AluOpType.add, op1=mybir.AluOpType.mult)
                t4 = wp.tile([BLK, BLK], F32, tag="t4")
                nc.vector.tensor_tensor(out=t4[:], in0=u1[:],
                                        in1=gps[:, 2 * H:3 * H],
                                        op=mybir.AluOpType.add)
                hhT = wp.tile([BLK, BLK], BF16, tag="hhT")
                nc.scalar.activation(hhT[:], t4[:],
                                     mybir.ActivationFunctionType.Tanh,
                                     bias=bkc[:, 2:3])
                d_t = wp.tile([BLK, BLK], BF16, tag="d_t")
                nc.vector.tensor_tensor(out=d_t[:], in0=hslice, in1=hhT[:],
                                        op=mybir.AluOpType.subtract)
                p2 = wp.tile([BLK, BLK], BF16, tag="p2")
                nc.vector.scalar_tensor_tensor(
                    out=p2[:], in0=zT[:], scalar=1.0, in1=d_t[:],
                    op0=mybir.AluOpType.add, op1=mybir.AluOpType.mult)
                nc.vector.scalar_tensor_tensor(
                    out=hT[:, bass.ts(b, BLK)], in0=p2[:], scalar=0.5,
                    in1=hhT[:], op0=mybir.AluOpType.mult,
                    op1=mybir.AluOpType.add)
                if t < t_iters - 1:
                    ac_block(b, t + 1)

            # --- prologue: AC from h0, first AllGathers ---
            for b in range(NBLK):
                ac_block(b, 0)

            def iteration(t):
                for h in range(2):
                    rep = repLo[t % 2] if h == 0 else repHi[t % 2]
                    for gi in range(NGRP):
                        t0g, ngt, calls = gmeta[(h, gi)]
                        if ngt == 0:
                            continue
                        gslot0 = t0g * BLK
                        gm = gmp.tile([BLK, ngt * 256], F8, tag="gm")
                        off = 0
                        for (_h, c0, take, _gi) in calls:
                            out_ap = gm[:, off * 2:(off + take) * 2].rearrange(
                                "p (k e) -> p k e", e=256)
                            nc.gpsimd.dma_gather(
                                out_ap, rep[:],
                                idx_sb[:, c0 // 16:(c0 + take) // 16],
                                take, take, 256, elem_step=256)
                            off += take
                        indT_sb = indp.tile([BLK, ngt * BLK], F8, tag="indT")
                        nc.sync.dma_start(
                            indT_sb[:], indT_in[:, gslot0:gslot0 + ngt * BLK])
                        indm_sb = indp.tile([BLK, ngt * BLK], F8, tag="indm")
                        nc.sync.dma_start(
                            indm_sb[:], indm_in[:, gslot0:gslot0 + ngt * BLK])

                        blk_first = {}
                        blk_last = {}
                        for k in range(ngt):
                            tb = int(tile_block[t0g + k])
                            if tb not in blk_first:
                                blk_first[tb] = k
                            blk_last[tb] = k
                        aggof = {}
                        u = 0
                        while u < ngt:
                            w = min(4, ngt - u)
                            xps = psX.tile([BLK, 4 * BLK], F32, space="PSUM",
                                           tag="x")
                            for k in range(w):
                                lt = u + k
                                ti = t0g + lt
                                tb = int(tile_block[ti])
                                par = int(tile_par[ti])
                                asl = slice(lt * 256 + par * 128,
                                            lt * 256 + par * 128 + 128)
                                nc.tensor.matmul(
                                    xps[:, bass.ts(k, BLK)],
                                    lhsT=ident8[:], rhs=gm[:, asl],
                                    start=True, stop=False)
                                nc.tensor.matmul(
                                    xps[:, bass.ts(k, BLK)],
                                    lhsT=indT_sb[:, bass.ts(lt, BLK)],
                                    rhs=c_all[:, bass.ts(tb, H)],
                                    start=False, stop=True)
                            e_sb = spool.tile([BLK, 4 * BLK], BF16, tag="e")
                            nc.scalar.activation(
                                e_sb[:, :w * BLK], xps[:, :w * BLK],
                                mybir.ActivationFunctionType.Exp,
                                bias=lnla_c[:, :1], scale=1.0)
                            r_sb = spool.tile([BLK, 4 * BLK], BF16, tag="r")
                            # relu alternates ACT/DVE per quad: ACT is the
                            # bottleneck engine, DVE has headroom
                            if (u // 4) % 2 == 0:
                                nc.scalar.activation(
                                    r_sb[:, :w * BLK], xps[:, :w * BLK],
                                    mybir.ActivationFunctionType.Relu,
                                    scale=LAM)
                            else:
                                nc.vector.tensor_scalar(
                                    out=r_sb[:, :w * BLK],
                                    in0=xps[:, :w * BLK],
                                    scalar1=LAM, scalar2=0.0,
                                    op0=mybir.AluOpType.mult,
                                    op1=mybir.AluOpType.max)
                            b_sb = spool.tile([BLK, 4 * BLK], BF16, tag="b")
                            nc.vector.tensor_scalar(
                                out=b_sb[:, :w * BLK], in0=e_sb[:, :w * BLK],
                                scalar1=LA, scalar2=0.0,
                                op0=mybir.AluOpType.subtract,
                                op1=mybir.AluOpType.min)
                            for k in range(w):
                                lt = u + k
                                ti = t0g + lt
                                tb = int(tile_block[ti])
                                if blk_first[tb] == lt:
                                    agg_t = psA.tile([BLK, BLK], F32,
                                                     space="PSUM", tag="agg")
                                    aggof[tb] = agg_t
                                    if h == 1:
                                        nc.tensor.matmul(
                                            agg_t[:], lhsT=ident_b[:],
                                            rhs=partial[:, bass.ts(tb, H)],
                                            start=True, stop=False)
                                        first_mm = False
                                    else:
                                        first_mm = True
                                else:
                                    first_mm = False
                                nc.tensor.matmul(
                                    aggof[tb][:],
                                    lhsT=r_sb[:, bass.ts(k, BLK)],
                                    rhs=indm_sb[:, bass.ts(lt, BLK)],
                                    start=first_mm, stop=False)
                                last = blk_last[tb] == lt
                                nc.tensor.matmul(
                                    aggof[tb][:],
                                    lhsT=b_sb[:, bass.ts(k, BLK)],
                                    rhs=indm_sb[:, bass.ts(lt, BLK)],
                                    start=False, stop=last)
                                if last:
                                    if h == 0:
                                        nc.vector.tensor_copy(
                                            partial[:, bass.ts(tb, H)],
                                            aggof[tb][:])
                                    else:
                                        gru_block(tb, aggof[tb][:], t)
                            u += w

            for t in range(t_iters):
                iteration(t)

            # --- pooling ---
            pool_ps = psX.tile([BLK, G], F32, space="PSUM", tag="x")
            for b in range(NBLK):
                gps = psG.tile([BLK, 4 * H], F32, space="PSUM", tag="gru")
                nc.tensor.matmul(gps[:, 0:H], lhsT=hT[:, bass.ts(b, BLK)],
                                 rhs=ident_b[:], start=True, stop=True)
                hb_sb = fp.tile([BLK, BLK], BF16, tag="hb_sb")
                nc.vector.tensor_copy(hb_sb[:], gps[:, 0:H])
                indg = fp.tile([BLK, G], BF16, tag="indg")
                nc.vector.tensor_scalar(
                    out=indg[:], in0=iota_g[:], scalar1=gid_sb[:, b:b + 1],
                    scalar2=None, op0=mybir.AluOpType.is_equal)
                nc.tensor.matmul(pool_ps[:], lhsT=hb_sb[:], rhs=indg[:],
                                 start=(b == 0), stop=(b == NBLK - 1))
            pooledT = fp.tile([BLK, G], F32, tag="pooledT")
            nc.vector.tensor_copy(pooledT[:], pool_ps[:])
            nc.sync.dma_start(pool_in[:], pooledT[:])
            nc.gpsimd.collective_compute(
                "AllReduce", mybir.AluOpType.add,
                replica_groups=AG_GROUPS,
                ins=[pool_in.opt()], outs=[pool_out.opt()])
            pld_f = fp.tile([BLK, G], F32, tag="pooledT")
            nc.sync.dma_start(pld_f[:], pool_out[:])
            pld = fp.tile([BLK, G], BF16, tag="pld")
            nc.vector.tensor_copy(pld[:], pld_f[:])

            def selu_block(x_ps, brel_col, bexp_col, tagp):
                rr = fp.tile([BLK, G], BF16, tag="f_r")
                nc.scalar.activation(rr[:], x_ps[:],
                                     mybir.ActivationFunctionType.Relu,
                                     bias=brel_col, scale=LAM)
                ee = fp.tile([BLK, G], BF16, tag="f_e")
                nc.scalar.activation(ee[:], x_ps[:],
                                     mybir.ActivationFunctionType.Exp,
                                     bias=bexp_col, scale=1.0)
                bb = fp.tile([BLK, G], BF16, tag="f_b")
                nc.vector.tensor_scalar(
                    out=bb[:], in0=ee[:], scalar1=LA, scalar2=0.0,
                    op0=mybir.AluOpType.subtract, op1=mybir.AluOpType.min)
                oo = fp.tile([BLK, G], BF16, tag=f"{tagp}_o")
                nc.vector.tensor_tensor(out=oo[:], in0=rr[:], in1=bb[:],
                                        op=mybir.AluOpType.add)
                return oo

            x1 = []
            for half in range(2):
                x_ps = psX.tile([BLK, G], F32, space="PSUM", tag="x")
                nc.tensor.matmul(x_ps[:], lhsT=w1[:, bass.ts(half, BLK)],
                                 rhs=pld[:], start=True, stop=True)
                x1.append(selu_block(x_ps, b1r[:, half:half + 1],
                                     b1e[:, half:half + 1], f"x1{half}"))
            x2 = []
            for half in range(2):
                x_ps = psX.tile([BLK, G], F32, space="PSUM", tag="x")
                nc.tensor.matmul(x_ps[:], lhsT=w2q[0][half][:], rhs=x1[0][:],
                                 start=True, stop=False)
                nc.tensor.matmul(x_ps[:], lhsT=w2q[1][half][:], rhs=x1[1][:],
                                 start=False, stop=True)
                x2.append(selu_block(x_ps, b2r[:, half:half + 1],
                                     b2e[:, half:half + 1], f"x2{half}"))
            x3_ps = psG.tile([BLK, 4 * H], F32, space="PSUM", tag="gru")
            nc.tensor.matmul(x3_ps[0:1, 0:G], lhsT=w3a[:], rhs=x2[0][:],
                             start=True, stop=False)
            nc.tensor.matmul(x3_ps[0:1, 0:G], lhsT=w3b[:], rhs=x2[1][:],
                             start=False, stop=True)
            out_sb = fp.tile([1, G], F32, tag="out_sb")
            nc.scalar.activation(out_sb[:], x3_ps[0:1, 0:G],
                                 mybir.ActivationFunctionType.Identity,
                                 bias=b3c[:1, :1])
            nc.sync.dma_start(out_dram[:], out_sb[:])

    nc.compile()
    return nc


def kernel(features, edges_topology, graph_ids, Wm, bm, Wk, Uk, bk,
           W1, b1, W2, b2, W3, b3, _trace=False, _t_iters=T_ITERS,
           _meta=None):
    features = np.asarray(features, np.float32)
    Wm = np.asarray(Wm, np.float32)
    bm = np.asarray(bm, np.float32)
    Wk = np.asarray(Wk, np.float32)
    Uk = np.asarray(Uk, np.float32).copy()
    Uk[:, 2 * H:3 * H] *= 0.5
    bk = np.asarray(bk, np.float32)
    W1 = np.asarray(W1, np.float32)
    b1 = np.asarray(b1, np.float32)
    W2 = np.asarray(W2, np.float32)
    b2 = np.asarray(b2, np.float32)
    W3 = np.asarray(W3, np.float32)
    b3 = np.asarray(b3, np.float32)
    et = np.asarray(edges_topology)

    meta = _meta
    if meta is None:
        meta = preprocess(features, et[0], et[1], graph_ids)
    nc = _build_program(meta, float(b3[0]), t_iters=_t_iters)

    bkc = np.stack(
        [
            0.5 * (bk[0, 0:H] + bk[1, 0:H]),
            0.5 * (bk[0, H:2 * H] + bk[1, H:2 * H]),
            bk[0, 2 * H:3 * H],
        ],
        axis=1,
    ).astype(np.float32)
    bhh2 = (0.5 * bk[1, 2 * H:3 * H]).reshape(1, H).astype(np.float32)

    b1r = np.stack([LAM * b1[0:BLK], LAM * b1[BLK:RU]], axis=1).astype(np.float32)
    b1e = np.stack([b1[0:BLK] + LNLA, b1[BLK:RU] + LNLA], axis=1).astype(np.float32)
    b2r = np.stack([LAM * b2[0:BLK], LAM * b2[BLK:RU]], axis=1).astype(np.float32)
    b2e = np.stack([b2[0:BLK] + LNLA, b2[BLK:RU] + LNLA], axis=1).astype(np.float32)

    in_maps = []
    for c in range(N_CORES):
        in_maps.append(
            {
                "h0T": np.asarray(meta["h0T"][c]),
                "idx16": meta["idx16"][c],
                "indT": meta["indT"][c],
                "indm": meta["indm"][c],
                "gid": meta["gid"][c],
                "wm1": np.ascontiguousarray(Wm[0:H]),
                "wm2": np.ascontiguousarray(Wm[H:2 * H]),
                "bmr": bm.reshape(1, H),
                "wk": Wk,
                "uk": Uk,
                "bhh2": bhh2,
                "bkc": bkc,
                "w1": W1,
                "w2": W2,
                "w3": W3,
                "b1r": b1r,
                "b1e": b1e,
                "b2r": b2r,
                "b2e": b2e,
            }
        )

    res = bass_utils.run_bass_kernel_spmd(
        nc, in_maps, core_ids=list(range(N_CORES)), trace=_trace
    )
    out = res.results[0]["out"].reshape(G, 1).astype(np.float32)
    kernel.last_results = res
    kernel.last_nc = nc
    return out
